# revision 16
# baseline (speedup 1.0000x reference)
"""Distributed Trainium2 kernel for a dense transformer block.

Sequence-parallel over 8 NeuronCores (512 tokens each; cores 0-3 hold
batch 0, 4-7 batch 1). Weights replicated. Two AllGathers (K then V)
within each 4-core group; everything else local.

Host-side preprocessing (in kernel()):
 - LayerNorm gains folded into the weights: Wqkv' = diag(ln1_g)@Wqkv,
   bqkv' = bqkv + ln1_b@Wqkv (same for W1/ln2), so the device only
   computes (x-mu)*rsqrt(var+eps).
 - Q/K columns permuted so the QKV matmul emits q/k directly in the
   [32-partition, dh-half-pair] layout DoubleRow score matmuls need.
 - Wqkv/Wo cast to fp8e4m3, W1/W2 to bf16 (attention path fp8 is
   ~1.9e-3 rel err; MLP must stay bf16).

Device-side structure:
 - All attention-path matmuls (QKV, scores, AV, Wo) run fp8 DoubleRow
   (two contraction tiles per instruction).
 - AV emits attn^T [dh, q] directly with a ones-column producing the
   softmax denominator; normalization happens before Wo via a
   partition-broadcast reciprocal. No DMA transposes.
 - exp and the hand-rolled tanh-form gelu share one ACT table
   (exp_and_others); LayerNorm rsqrt is Newton iteration on DVE, so
   the scalar engine never swaps tables after warmup.
 - Tokens are processed in two 256-token chunks: chunk1's softmax exp
   (scalar engine) overlaps chunk0's MLP (tensor engine) via
   interleaved emission.
"""

import sys

if "/opt/trn_rl_repo" not in sys.path:
    sys.path.insert(0, "/opt/trn_rl_repo")

import numpy as np

B, S, D = 2, 2048, 1024
H, DH, FF = 16, 64, 4096
NCORES = 8
TOK = (B * S) // NCORES      # 512 tokens per core
P = 128
TT = TOK // P                # 4 token tiles
KD = D // P                  # 8 contract tiles over D
FT = FF // P                 # 32 tiles over FF
GS = 4                       # group size (cores per batch)
NKJ = S // P                 # 16 key tiles per batch
CH = 2                       # token chunks for attn/MLP pipelining
CQT = TT // CH               # 2 token tiles per chunk
CTOK = TOK // CH             # 256 tokens per chunk
GROUPS = [[0, 1, 2, 3], [4, 5, 6, 7]]
KEL = D * TOK                # fp8 elements in K bounce buffer
VEL = D * TOK

_cache = {}


def _build():
    from contextlib import ExitStack
    from concourse import bacc, tile, mybir
    from concourse.masks import make_identity

    F32 = mybir.dt.float32
    BF16 = mybir.dt.bfloat16
    F8 = mybir.dt.float8e4
    Alu = mybir.AluOpType
    Act = mybir.ActivationFunctionType
    DR = mybir.MatmulPerfMode.DoubleRow

    nc = bacc.Bacc("TRN2", target_bir_lowering=False, debug=False,
                   num_devices=NCORES)

    x_ext = nc.dram_tensor("x", [TOK, D], F32, kind="ExternalInput")
    wqkv8 = nc.dram_tensor("wqkv8", [D, 3 * D], F8, kind="ExternalInput")
    bqkvf = nc.dram_tensor("bqkvf", [3 * D], F32, kind="ExternalInput")
    wo8_ext = nc.dram_tensor("wo8", [D, D], F8, kind="ExternalInput")
    bo_ext = nc.dram_tensor("bo", [D], F32, kind="ExternalInput")
    w1_ext = nc.dram_tensor("w1f", [D, FF], BF16, kind="ExternalInput")
    b1_ext = nc.dram_tensor("b1f", [FF], F32, kind="ExternalInput")
    w2_ext = nc.dram_tensor("w2b", [FF, D], BF16, kind="ExternalInput")
    b2_ext = nc.dram_tensor("b2", [D], F32, kind="ExternalInput")
    out_ext = nc.dram_tensor("out", [TOK, D], F32, kind="ExternalOutput")

    with tile.TileContext(nc) as tc, ExitStack() as ctx:
        const = ctx.enter_context(tc.tile_pool(name="const", bufs=1))
        persist = ctx.enter_context(tc.tile_pool(name="persist", bufs=1))
        wq = ctx.enter_context(tc.tile_pool(name="wq", bufs=6))
        wv_p = ctx.enter_context(tc.tile_pool(name="wv_p", bufs=2))
        shp = ctx.enter_context(tc.tile_pool(name="shp", bufs=2))
        shw = ctx.enter_context(tc.tile_pool(name="shw", bufs=2))
        wmlp = ctx.enter_context(tc.tile_pool(name="wmlp", bufs=3))
        act = ctx.enter_context(tc.tile_pool(name="act", bufs=3))
        probs = ctx.enter_context(tc.tile_pool(name="probs", bufs=4))
        gelu = ctx.enter_context(tc.tile_pool(name="gelu", bufs=2))
        norm = ctx.enter_context(tc.tile_pool(name="norm", bufs=2))
        mm_ps = ctx.enter_context(
            tc.tile_pool(name="mm_ps", bufs=3, space="PSUM"))
        sp_ps = ctx.enter_context(
            tc.tile_pool(name="sp_ps", bufs=2, space="PSUM"))
        av_ps = ctx.enter_context(
            tc.tile_pool(name="av_ps", bufs=1, space="PSUM"))
        dram = ctx.enter_context(tc.tile_pool(name="dram", bufs=1, space="DRAM"))

        sy, gp, ve, sc, te = nc.sync, nc.gpsimd, nc.vector, nc.scalar, nc.tensor

        # x lands first on the sync queue
        x1_sb = persist.tile([P, TT, D], F32, tag="x1")
        for t in range(TT):
            sy.dma_start(x1_sb[:, t, :], x_ext[t * P:(t + 1) * P, :])

        # ---------------- constants ----------------
        ones_row = const.tile([1, P], BF16)
        ve.memset(ones_row[:], 1.0)
        ident = const.tile([P, P], BF16)
        make_identity(nc, ident[:])

        # prewarm the exp/tanh ACT table off the critical path
        warm = const.tile([1, 2], F32)
        ve.memset(warm[:], 0.0)
        sc.activation(warm[:, 1:2], warm[:, 0:1], Act.Exp)

        bqkv_qk = const.tile([P, 16], F32)
        gp.dma_start(bqkv_qk[:], bqkvf[0:2 * D].rearrange("(m p) -> p m", p=P))
        b1col = const.tile([P, FT], F32)
        gp.dma_start(b1col[:], b1_ext[:].rearrange("(m p) -> p m", p=P))

        def brow(src, name, dt=F32):
            row = act.tile([1, D], F32, tag="crow", name=f"{name}_r")
            gp.dma_start(row[:], src.rearrange("(a d) -> a d", a=1))
            if dt == F32:
                return row
            rb = const.tile([1, D], dt, name=name)
            ve.tensor_copy(rb[:], row[:])
            return rb

        b2row = brow(b2_ext[:], "b2row", BF16)
        bo_row = brow(bo_ext[:], "bo_row")
        bv_row = brow(bqkvf[2 * D:3 * D], "bv_row")
        bo_bc = const.tile([P, D], F32, name="bo_bc")
        gp.partition_broadcast(bo_bc[:], bo_row[:])
        bv_bc = const.tile([P, D], F32, name="bv_bc")
        gp.partition_broadcast(bv_bc[:], bv_row[:])

        # Wo in [64-part, head, dout] layout for DoubleRow head pairs
        wo8t = persist.tile([64, H, D], F8, tag="wo8t")
        gp.dma_start(wo8t[:], wo8_ext[:, :].rearrange("(h p) c -> p h c", p=64))

        # ---------------- helpers ----------------
        def newton_rsqrt(y_ap, v_ap, tmp_ap):
            # y = rsqrt(v), v in ~[0.7, 1.4]: 3 Newton steps from y0=1
            ve.memset(y_ap, 1.0)
            for _ in range(3):
                ve.tensor_mul(tmp_ap, y_ap, y_ap)
                ve.tensor_mul(tmp_ap, tmp_ap, v_ap)
                ve.tensor_scalar(tmp_ap, tmp_ap, scalar1=-0.5, scalar2=1.5,
                                 op0=Alu.mult, op1=Alu.add)
                ve.tensor_mul(y_ap, y_ap, tmp_ap)

        def layer_norm(x_ap, out_ap):
            st = act.tile([P, 2, 6], F32, tag="ln_st", name="ln_st")
            ve.bn_stats(st[:, 0, :], x_ap[:, 0:512])
            ve.bn_stats(st[:, 1, :], x_ap[:, 512:1024])
            mv = act.tile([P, 2], F32, tag="ln_mv", name="ln_mv")
            ve.bn_aggr(mv[:], st[:])
            vv = act.tile([P, 3], F32, tag="ln_vy", name="ln_vy")
            ve.tensor_scalar_add(vv[:, 0:1], mv[:, 1:2], 1e-5)
            newton_rsqrt(vv[:, 1:2], vv[:, 0:1], vv[:, 2:3])
            ve.tensor_scalar(out_ap, x_ap, scalar1=mv[:, 0:1],
                             scalar2=vv[:, 1:2], op0=Alu.subtract,
                             op1=Alu.mult)

        def pe_t(dst_ap, src_ap):
            tp = mm_ps.tile([P, P], BF16, tag="mm", name="tp")
            te.transpose(tp[:], src_ap, ident[:])
            ve.tensor_copy(dst_ap, tp[:])

        # ---------------- phase 1: LN1 + transpose (fp8 hT) ----------
        # hT shares a 4KB/partition ring with the later batched W2 tiles
        hT = shw.tile([P, KD, TOK], F8, tag="shw", name="hT")
        for t in range(TT):
            ht = act.tile([P, D], BF16, tag="hmt", name="hmt")
            layer_norm(x1_sb[:, t, :], ht[:])
            for k in range(KD):
                pe_t(hT[:, k, t * P:(t + 1) * P], ht[:, k * P:(k + 1) * P])
            # residual absorbs the Wo bias once, after LN1 consumed x
            ve.tensor_add(x1_sb[:, t, :], x1_sb[:, t, :], bo_bc[:])

        # ---------------- phase 2: K, AG(K), Q, V, AG(V) --------------
        qT = persist.tile([P, KD, TOK], F8, tag="qT")
        # kTl shares a 4KB/partition ring with the later mT chunk tiles
        kTl = shp.tile([P, KD, TOK], F8, tag="sh", name="kTl")

        def qk_tile(m, per_t):
            # one 128-column tile of the (permuted) Q/K projection
            wqt = wq.tile([P, KD, P], F8, tag="wq", name="wq")
            gp.dma_start(wqt[:], wqkv8[:, m * P:(m + 1) * P].rearrange(
                "(k p) m -> p k m", p=P))
            ps = mm_ps.tile([P, TOK], F32, tag="mm", name="mm_qk")
            tranges = [(t * P, (t + 1) * P) for t in range(TT)] if per_t \
                else [(0, TOK)]
            for k2 in range(KD // 2):
                for lo, hi in tranges:
                    te.matmul(ps[:, lo:hi], wqt[:, 2 * k2:2 * k2 + 2, :],
                              hT[:, 2 * k2:2 * k2 + 2, lo:hi],
                              start=(k2 == 0), stop=(k2 == KD // 2 - 1),
                              perf_mode=DR)
            dst = qT if m < 8 else kTl
            ve.tensor_scalar_add(dst[:, m % 8, :], ps[:],
                                 scalar1=bqkv_qk[:, m:m + 1])

        for m in range(8, 16):       # K first (per-t chases LN1 tiles)
            qk_tile(m, per_t=True)

        # V in natural token-major layout
        v_sb = persist.tile([P, TT, D], F8, tag="v_sb")
        for c in range(2):
            wvt = wv_p.tile([P, KD, 512], F8, tag="wv", name="wv")
            gp.dma_start(wvt[:], wqkv8[:, 2 * D + c * 512:2 * D + (c + 1) * 512]
                         .rearrange("(k p) m -> p k m", p=P))
            for t in range(TT):
                ps = mm_ps.tile([P, 512], F32, tag="mm", name="mm_v")
                for k2 in range(KD // 2):
                    te.matmul(ps[:], hT[:, 2 * k2:2 * k2 + 2,
                                        t * P:(t + 1) * P],
                              wvt[:, 2 * k2:2 * k2 + 2, :],
                              start=(k2 == 0), stop=(k2 == KD // 2 - 1),
                              perf_mode=DR)
                ve.tensor_add(v_sb[:, t, c * 512:(c + 1) * 512], ps[:],
                              bv_bc[:, c * 512:(c + 1) * 512])

        # single AllGather of K^T and V, launched as soon as both exist
        CCIN = 2 * KEL
        cc_in = dram.tile([CCIN], F8)
        gp.dma_start(cc_in[0:KEL].rearrange("(k p t) -> p k t", k=KD, p=P),
                     kTl[:])
        gp.dma_start(cc_in[KEL:CCIN].rearrange("(t p d) -> p t d", t=TT, p=P),
                     v_sb[:])
        cc_out = dram.tile([GS * CCIN], F8)
        gp.collective_compute(
            "AllGather", Alu.bypass, ins=[cc_in[:]], outs=[cc_out[:]],
            replica_groups=GROUPS)

        for m in range(0, 8):        # Q overlaps the ring
            qk_tile(m, per_t=False)

        # unpack K (two queues), then V
        kT_full = persist.tile([P, KD, GS, TOK], F8, tag="kT_full")
        for r in range(GS):
            eng = sy if r % 2 == 0 else gp
            eng.dma_start(
                kT_full[:, :, r, :],
                cc_out[r * CCIN:r * CCIN + KEL].rearrange(
                    "(k p t) -> p k t", k=KD, p=P))
        v_aug = persist.tile([P, NKJ, H, 65], F8, tag="v_aug")
        ve.memset(v_aug[:, :, :, 64:65], 1.0)
        for r in range(GS):
            for vt in range(TT):
                eng = sy if (r * TT + vt) % 2 == 0 else gp
                base = r * CCIN + KEL + vt * P * D
                eng.dma_start(
                    v_aug[:, r * TT + vt, :, 0:64],
                    cc_out[base:base + P * D].rearrange(
                        "(p h f) -> p h f", p=P, h=H))

        # ------- phase 3: chunked attention + interleaved MLP ---------
        attnT = persist.tile([64, H, TOK], F8, tag="attnT")
        mT = [shp.tile([P, KD, CTOK], BF16, tag="sh", name=f"mT{c}")
              for c in range(CH)]
        g1T = persist.tile([P, FT, CTOK], BF16, tag="g1T")

        def attention_chunk(ch, units):
            q0 = ch * CTOK
            done = [0]
            step = [0]
            nsteps = 8 * (NKJ // 2)

            def pump():
                step[0] += 1
                want = (step[0] * len(units)) // nsteps if units else 0
                while done[0] < want:
                    units[done[0]]()
                    done[0] += 1

            pend = []   # (av, pb, j2) awaiting AV emission
            for pr in range(8):
                av = av_ps.tile([65, 2, CTOK], F32, tag="av", name="av")
                b32 = (pr % 2) * 64
                mt0 = 2 * (pr // 2)
                for j2 in range(NKJ // 2):
                    r, jj = divmod(2 * j2, TT)
                    sp = sp_ps.tile([P, 2, 2, CTOK], F32, tag="sp", name="sp")
                    for hpp in range(2):
                        bb = b32 + hpp * 32
                        for dj in range(2):
                            te.matmul(
                                sp[:, hpp, dj, :],
                                kT_full[bb:bb + 32, mt0:mt0 + 2, r,
                                        (jj + dj) * P:(jj + dj + 1) * P],
                                qT[bb:bb + 32, mt0:mt0 + 2, q0:q0 + CTOK],
                                start=True, stop=True, perf_mode=DR,
                                tile_position=(bb, 0))
                    pb = probs.tile([P, 2, 2, CTOK], F8, tag="probs",
                                    name="probs")
                    sc.activation(pb[:], sp[:], Act.Exp, scale=0.125)
                    pend.append((av, pb, j2))
                    if len(pend) > 1:
                        pav, ppb, pj2 = pend.pop(0)
                        for h2 in range(2):
                            te.matmul(pav[:, h2, :],
                                      v_aug[:, 2 * pj2:2 * pj2 + 2,
                                            2 * pr + h2, :],
                                      ppb[:, h2, :, :],
                                      start=(pj2 == 0), stop=False,
                                      perf_mode=DR)
                    pump()
                # last AV of the pair
                pav, ppb, pj2 = pend.pop(0)
                for h2 in range(2):
                    te.matmul(pav[:, h2, :],
                              v_aug[:, 2 * pj2:2 * pj2 + 2, 2 * pr + h2, :],
                              ppb[:, h2, :, :],
                              start=(pj2 == 0), stop=(pj2 == 7),
                              perf_mode=DR)
                # normalize: reciprocal of the ones-column accumulator
                dstg = norm.tile([65, 2, CTOK], F32, tag="dstg", name="dstg")
                ve.tensor_copy(dstg[64:65, :, :], av[64:65, :, :])
                den0 = norm.tile([1, 2, 2, CTOK], F32, tag="den0", name="den0")
                gp.dma_start(den0[:, 0, :, :], dstg[64:65, :, :])
                ve.reciprocal_approx_fast(den0[:, 1, :, :], den0[:, 0, :, :])
                rbc = norm.tile([64, 2, CTOK], F32, tag="rbc", name="rbc")
                gp.partition_broadcast(rbc[:], den0[:, 1, :, :])
                for h2 in range(2):
                    ve.tensor_mul(attnT[:, 2 * pr + h2, q0:q0 + CTOK],
                                  av[0:64, h2, :], rbc[:, h2, :])
            while done[0] < len(units):
                units[done[0]]()
                done[0] += 1

        def wo_chunk(ch):
            q0 = ch * CTOK
            for c2 in range(2):
                for qt in range(CQT):
                    ps = mm_ps.tile([P, 512], F32, tag="mm", name="mm_wo")
                    for pr in range(8):
                        te.matmul(ps[:],
                                  attnT[:, 2 * pr:2 * pr + 2,
                                        q0 + qt * P:q0 + (qt + 1) * P],
                                  wo8t[:, 2 * pr:2 * pr + 2,
                                       c2 * 512:(c2 + 1) * 512],
                                  start=(pr == 0), stop=(pr == 7),
                                  perf_mode=DR)
                    xsl = x1_sb[:, ch * CQT + qt, c2 * 512:(c2 + 1) * 512]
                    ve.tensor_add(xsl, xsl, ps[:])

        def ln2_chunk(ch):
            for qt in range(CQT):
                mt = act.tile([P, D], BF16, tag="hmt", name="mln")
                layer_norm(x1_sb[:, ch * CQT + qt, :], mt[:])
                for k in range(KD):
                    pe_t(mT[ch][:, k, qt * P:(qt + 1) * P],
                         mt[:, k * P:(k + 1) * P])

        def mlp_units(ch):
            units = []
            c = 0.7978845608028654

            def w1_unit(mp):
                def u():
                    # gelu element ops alternate DVE / gpsimd per block
                    ew = ve if mp % 2 == 0 else gp
                    w1c = wmlp.tile([P, KD, 2 * P], BF16, tag="w1c",
                                    name="w1c")
                    gp.dma_start(w1c[:], w1_ext[:, mp * 2 * P:(mp + 1) * 2 * P]
                                 .rearrange("(k p) m -> p k m", p=P))
                    ps = mm_ps.tile([P, 2, CTOK], F32, tag="mm", name="mm_w1")
                    for hf in range(2):
                        for k in range(KD):
                            te.matmul(ps[:, hf, :],
                                      w1c[:, k, hf * P:(hf + 1) * P],
                                      mT[ch][:, k, :],
                                      start=(k == 0), stop=(k == KD - 1))
                    xb = gelu.tile([P, 2, CTOK], BF16, tag="gxb", name="gxb")
                    for hf in range(2):
                        # PSUM read must stay on DVE (gpsimd can't touch PSUM)
                        ve.tensor_scalar_add(
                            xb[:, hf, :], ps[:, hf, :],
                            scalar1=b1col[:, 2 * mp + hf:2 * mp + hf + 1])
                    t1 = gelu.tile([P, 2, CTOK], BF16, tag="gt1", name="gt1")
                    ew.tensor_mul(t1[:], xb[:], xb[:])
                    ew.tensor_scalar(t1[:], t1[:], scalar1=0.044715,
                                     scalar2=1.0, op0=Alu.mult, op1=Alu.add)
                    ew.tensor_mul(t1[:], t1[:], xb[:])
                    th = gelu.tile([P, 2, CTOK], BF16, tag="gth", name="gth")
                    sc.activation(th[:], t1[:], Act.Tanh, scale=c)
                    ew.tensor_scalar(th[:], th[:], scalar1=0.5, scalar2=0.5,
                                     op0=Alu.mult, op1=Alu.add)
                    ew.tensor_mul(g1T[:, 2 * mp:2 * mp + 2, :], th[:], xb[:])
                return u

            for mp in range(FT // 2):
                units.append(w1_unit(mp))

            w2state = {}

            def w2_unit(c2, fg):
                def u():
                    if fg == 0:
                        w2state[c2] = [
                            mm_ps.tile([P, 512], F32, tag="mm",
                                       name="mm_w2")
                            for _ in range(CQT)]
                    pss = w2state[c2]
                    # one batched load covers 4 FF row-tiles
                    w2c = shw.tile([P, 4, 512], BF16, tag="shw", name="w2c")
                    f0 = fg * 4
                    sy.dma_start(w2c[:], w2_ext[f0 * P:(f0 + 4) * P,
                                                c2 * 512:(c2 + 1) * 512]
                                 .rearrange("(f p) c -> p f c", p=P))
                    for fi in range(4):
                        for qt in range(CQT):
                            te.matmul(pss[qt][:],
                                      g1T[:, f0 + fi, qt * P:(qt + 1) * P],
                                      w2c[:, fi, :],
                                      start=(f0 + fi == 0), stop=False)
                    if fg == 7:
                        for qt in range(CQT):
                            te.matmul(pss[qt][:], ones_row[:],
                                      b2row[:, c2 * 512:(c2 + 1) * 512],
                                      start=False, stop=True)
                            ot = act.tile([P, 512], F32, tag="oout",
                                          name="oout")
                            ve.tensor_add(
                                ot[:], pss[qt][:],
                                x1_sb[:, ch * CQT + qt,
                                      c2 * 512:(c2 + 1) * 512])
                            row = (ch * CQT + qt) * P
                            sy.dma_start(
                                out_ext[row:row + P,
                                        c2 * 512:(c2 + 1) * 512], ot[:])
                return u

            for c2 in range(2):
                for fg in range(8):
                    units.append(w2_unit(c2, fg))
            return units

        attention_chunk(0, [])
        wo_chunk(0)
        ln2_chunk(0)
        attention_chunk(1, mlp_units(0))
        wo_chunk(1)
        ln2_chunk(1)
        for u in mlp_units(1):
            u()

    nc.compile()
    return nc


def _get_nc():
    if "nc" not in _cache:
        _cache["nc"] = _build()
    return _cache["nc"]


def _prep_weights(inputs):
    import ml_dtypes
    F8 = ml_dtypes.float8_e4m3
    BF = ml_dtypes.bfloat16

    f = {k: np.asarray(inputs[k], dtype=np.float32) for k in (
        "ln1_g", "ln1_b", "Wqkv", "bqkv", "Wo", "bo",
        "ln2_g", "ln2_b", "W1", "b1", "W2", "b2")}

    Wqkv = f["ln1_g"][:, None] * f["Wqkv"]
    bqkv = f["bqkv"] + f["ln1_b"] @ f["Wqkv"]
    W1 = f["ln2_g"][:, None] * f["W1"]
    b1 = f["b1"] + f["ln2_b"] @ f["W1"]

    # Q/K column permutation for DoubleRow score layout
    d = np.arange(D)
    pr, rem = d // 128, d % 128
    hp, half, p32 = rem // 64, (rem % 64) // 32, d % 32
    newpos = (pr // 2 * 2 + half) * 128 + (pr % 2) * 64 + hp * 32 + p32
    wq_p = np.empty((D, 3 * D), np.float32)
    wq_p[:, newpos] = Wqkv[:, :D]
    wq_p[:, D + newpos] = Wqkv[:, D:2 * D]
    wq_p[:, 2 * D:] = Wqkv[:, 2 * D:]
    bq_p = np.empty(3 * D, np.float32)
    bq_p[newpos] = bqkv[:D]
    bq_p[D + newpos] = bqkv[D:2 * D]
    bq_p[2 * D:] = bqkv[2 * D:]

    return {
        "wqkv8": np.ascontiguousarray(wq_p.astype(F8)),
        "bqkvf": np.ascontiguousarray(bq_p),
        "wo8": np.ascontiguousarray(f["Wo"].astype(F8)),
        "bo": f["bo"],
        "w1f": np.ascontiguousarray(W1.astype(BF)),
        "b1f": np.ascontiguousarray(b1),
        "w2b": np.ascontiguousarray(f["W2"].astype(BF)),
        "b2": f["b2"],
    }


def make_in_maps(inputs):
    x = np.ascontiguousarray(np.asarray(inputs["x"], dtype=np.float32))
    flat = x.reshape(B * S, D)
    w = _prep_weights(inputs)
    in_maps = []
    for c in range(NCORES):
        m = {"x": np.ascontiguousarray(flat[c * TOK:(c + 1) * TOK])}
        m.update(w)
        in_maps.append(m)
    return in_maps


def kernel(**inputs):
    from concourse.bass_utils import run_bass_kernel_spmd

    nc = _get_nc()
    in_maps = make_in_maps(inputs)
    res = run_bass_kernel_spmd(nc, in_maps, core_ids=list(range(NCORES)))
    out = np.concatenate([res.results[c]["out"] for c in range(NCORES)],
                         axis=0)
    return out.reshape(B, S, D).astype(np.float32)


# revision 25
# speedup vs baseline: 1.0520x; 1.0520x over previous
"""Distributed Trainium2 kernel for a dense transformer block.

Sequence-parallel over 8 NeuronCores (512 tokens each; cores 0-3 hold
batch 0, 4-7 batch 1). Weights replicated. Two AllGathers (K then V)
within each 4-core group; everything else local.

Host-side preprocessing (in kernel()):
 - LayerNorm gains folded into the weights: Wqkv' = diag(ln1_g)@Wqkv,
   bqkv' = bqkv + ln1_b@Wqkv (same for W1/ln2), so the device only
   computes (x-mu)*rsqrt(var+eps).
 - Q/K columns permuted so the QKV matmul emits q/k directly in the
   [32-partition, dh-half-pair] layout DoubleRow score matmuls need.
 - Wqkv/Wo cast to fp8e4m3, W1/W2 to bf16 (attention path fp8 is
   ~1.9e-3 rel err; MLP must stay bf16).

Device-side structure:
 - All attention-path matmuls (QKV, scores, AV, Wo) run fp8 DoubleRow
   (two contraction tiles per instruction).
 - AV emits attn^T [dh, q] directly with a ones-column producing the
   softmax denominator; normalization happens before Wo via a
   partition-broadcast reciprocal. No DMA transposes.
 - exp and the hand-rolled tanh-form gelu share one ACT table
   (exp_and_others); LayerNorm rsqrt is Newton iteration on DVE, so
   the scalar engine never swaps tables after warmup.
 - Tokens are processed in two 256-token chunks: chunk1's softmax exp
   (scalar engine) overlaps chunk0's MLP (tensor engine) via
   interleaved emission.
"""

import sys

if "/opt/trn_rl_repo" not in sys.path:
    sys.path.insert(0, "/opt/trn_rl_repo")

import numpy as np

B, S, D = 2, 2048, 1024
H, DH, FF = 16, 64, 4096
NCORES = 8
TOK = (B * S) // NCORES      # 512 tokens per core
P = 128
TT = TOK // P                # 4 token tiles
KD = D // P                  # 8 contract tiles over D
FT = FF // P                 # 32 tiles over FF
GS = 4                       # group size (cores per batch)
NKJ = S // P                 # 16 key tiles per batch
CH = 2                       # token chunks for attn/MLP pipelining
CQT = TT // CH               # 2 token tiles per chunk
CTOK = TOK // CH             # 256 tokens per chunk
GROUPS = [[0, 1, 2, 3], [4, 5, 6, 7]]
KEL = D * TOK                # fp8 elements in K bounce buffer
VEL = D * TOK

_cache = {}


def _build():
    from contextlib import ExitStack
    from concourse import bacc, tile, mybir
    from concourse.masks import make_identity

    F32 = mybir.dt.float32
    BF16 = mybir.dt.bfloat16
    F8 = mybir.dt.float8e4
    Alu = mybir.AluOpType
    Act = mybir.ActivationFunctionType
    DR = mybir.MatmulPerfMode.DoubleRow

    nc = bacc.Bacc("TRN2", target_bir_lowering=False, debug=False,
                   num_devices=NCORES)

    x_ext = nc.dram_tensor("x", [TOK, D], F32, kind="ExternalInput")
    wqkv8 = nc.dram_tensor("wqkv8", [D, 3 * D], F8, kind="ExternalInput")
    bqkvf = nc.dram_tensor("bqkvf", [3 * D], F32, kind="ExternalInput")
    wo8_ext = nc.dram_tensor("wo8", [D, D], F8, kind="ExternalInput")
    bo_ext = nc.dram_tensor("bo", [D], F32, kind="ExternalInput")
    w1_ext = nc.dram_tensor("w1f", [D, FF], BF16, kind="ExternalInput")
    b1_ext = nc.dram_tensor("b1f", [FF], F32, kind="ExternalInput")
    w2_ext = nc.dram_tensor("w2b", [FF, D], BF16, kind="ExternalInput")
    b2_ext = nc.dram_tensor("b2", [D], F32, kind="ExternalInput")
    out_ext = nc.dram_tensor("out", [TOK, D], F32, kind="ExternalOutput")

    with tile.TileContext(nc) as tc, ExitStack() as ctx:
        const = ctx.enter_context(tc.tile_pool(name="const", bufs=1))
        persist = ctx.enter_context(tc.tile_pool(name="persist", bufs=1))
        wq = ctx.enter_context(tc.tile_pool(name="wq", bufs=6))
        wv_p = ctx.enter_context(tc.tile_pool(name="wv_p", bufs=2))
        shp = ctx.enter_context(tc.tile_pool(name="shp", bufs=2))
        shw = ctx.enter_context(tc.tile_pool(name="shw", bufs=2))
        wmlp = ctx.enter_context(tc.tile_pool(name="wmlp", bufs=3))
        act = ctx.enter_context(tc.tile_pool(name="act", bufs=3))
        probs = ctx.enter_context(tc.tile_pool(name="probs", bufs=3))
        gelu = ctx.enter_context(tc.tile_pool(name="gelu", bufs=2))
        norm = ctx.enter_context(tc.tile_pool(name="norm", bufs=1))
        mm_ps = ctx.enter_context(
            tc.tile_pool(name="mm_ps", bufs=3, space="PSUM"))
        sp_ps = ctx.enter_context(
            tc.tile_pool(name="sp_ps", bufs=2, space="PSUM"))
        av_ps = ctx.enter_context(
            tc.tile_pool(name="av_ps", bufs=1, space="PSUM"))
        dram = ctx.enter_context(tc.tile_pool(name="dram", bufs=1, space="DRAM"))

        sy, gp, ve, sc, te = nc.sync, nc.gpsimd, nc.vector, nc.scalar, nc.tensor

        # x lands first on the sync queue
        x1_sb = persist.tile([P, TT, D], F32, tag="x1")
        for t in range(TT):
            sy.dma_start(x1_sb[:, t, :], x_ext[t * P:(t + 1) * P, :])

        # ---------------- constants ----------------
        ones_row = const.tile([1, P], BF16)
        ve.memset(ones_row[:], 1.0)
        ident = const.tile([P, P], BF16)
        make_identity(nc, ident[:])

        # prewarm the exp/tanh ACT table off the critical path
        warm = const.tile([1, 2], F32)
        ve.memset(warm[:], 0.0)
        sc.activation(warm[:, 1:2], warm[:, 0:1], Act.Exp)

        bqkv_qk = const.tile([P, 16], F32)
        gp.dma_start(bqkv_qk[:], bqkvf[0:2 * D].rearrange("(m p) -> p m", p=P))
        b1col = const.tile([P, FT], F32)
        gp.dma_start(b1col[:], b1_ext[:].rearrange("(m p) -> p m", p=P))

        def brow(src, name, dt=F32):
            row = act.tile([1, D], F32, tag="crow", name=f"{name}_r")
            gp.dma_start(row[:], src.rearrange("(a d) -> a d", a=1))
            if dt == F32:
                return row
            rb = const.tile([1, D], dt, name=name)
            ve.tensor_copy(rb[:], row[:])
            return rb

        b2row = brow(b2_ext[:], "b2row", BF16)
        bo_row = brow(bo_ext[:], "bo_row")
        bv_row = brow(bqkvf[2 * D:3 * D], "bv_row")
        bo_bc = const.tile([P, D], F32, name="bo_bc")
        gp.partition_broadcast(bo_bc[:], bo_row[:])
        bv_bc = const.tile([P, D], F32, name="bv_bc")
        gp.partition_broadcast(bv_bc[:], bv_row[:])

        # Wo in [64-part, head, dout] layout for DoubleRow head pairs
        wo8t = persist.tile([64, H, D], F8, tag="wo8t")
        gp.dma_start(wo8t[:], wo8_ext[:, :].rearrange("(h p) c -> p h c", p=64))

        # ---------------- helpers ----------------
        def newton_rsqrt(y_ap, v_ap, tmp_ap):
            # y = rsqrt(v), v in ~[0.7, 1.4]: 3 Newton steps from y0=1
            ve.memset(y_ap, 1.0)
            for _ in range(3):
                ve.tensor_mul(tmp_ap, y_ap, y_ap)
                ve.tensor_mul(tmp_ap, tmp_ap, v_ap)
                ve.tensor_scalar(tmp_ap, tmp_ap, scalar1=-0.5, scalar2=1.5,
                                 op0=Alu.mult, op1=Alu.add)
                ve.tensor_mul(y_ap, y_ap, tmp_ap)

        def layer_norm(x_ap, out_ap):
            st = act.tile([P, 2, 6], F32, tag="ln_st", name="ln_st")
            ve.bn_stats(st[:, 0, :], x_ap[:, 0:512])
            ve.bn_stats(st[:, 1, :], x_ap[:, 512:1024])
            mv = act.tile([P, 2], F32, tag="ln_mv", name="ln_mv")
            ve.bn_aggr(mv[:], st[:])
            vv = act.tile([P, 3], F32, tag="ln_vy", name="ln_vy")
            ve.tensor_scalar_add(vv[:, 0:1], mv[:, 1:2], 1e-5)
            newton_rsqrt(vv[:, 1:2], vv[:, 0:1], vv[:, 2:3])
            ve.tensor_scalar(out_ap, x_ap, scalar1=mv[:, 0:1],
                             scalar2=vv[:, 1:2], op0=Alu.subtract,
                             op1=Alu.mult)

        def pe_t(dst_ap, src_ap):
            tp = mm_ps.tile([P, P], BF16, tag="mm", name="tp")
            te.transpose(tp[:], src_ap, ident[:])
            ve.tensor_copy(dst_ap, tp[:])

        # ---------------- phase 1: LN1 + transpose (fp8 hT) ----------
        hT = persist.tile([P, KD, TOK], F8, tag="hT")
        for t in range(TT):
            ht = act.tile([P, D], BF16, tag="hmt", name="hmt")
            layer_norm(x1_sb[:, t, :], ht[:])
            for k in range(KD):
                pe_t(hT[:, k, t * P:(t + 1) * P], ht[:, k * P:(k + 1) * P])
            # residual absorbs the Wo bias once, after LN1 consumed x
            ve.tensor_add(x1_sb[:, t, :], x1_sb[:, t, :], bo_bc[:])

        # ---------------- phase 2: K, AG(K), Q, V, AG(V) --------------
        qT = persist.tile([P, KD, TOK], F8, tag="qT")
        # kTl shares a 4KB/partition ring with the later mT chunk tiles
        kTl = shp.tile([P, KD, TOK], F8, tag="sh", name="kTl")

        def qk_tile(m, per_t):
            # one 128-column tile of the (permuted) Q/K projection
            wqt = wq.tile([P, KD, P], F8, tag="wq", name="wq")
            gp.dma_start(wqt[:], wqkv8[:, m * P:(m + 1) * P].rearrange(
                "(k p) m -> p k m", p=P))
            ps = mm_ps.tile([P, TOK], F32, tag="mm", name="mm_qk")
            tranges = [(t * P, (t + 1) * P) for t in range(TT)] if per_t \
                else [(0, TOK)]
            for k2 in range(KD // 2):
                for lo, hi in tranges:
                    te.matmul(ps[:, lo:hi], wqt[:, 2 * k2:2 * k2 + 2, :],
                              hT[:, 2 * k2:2 * k2 + 2, lo:hi],
                              start=(k2 == 0), stop=(k2 == KD // 2 - 1),
                              perf_mode=DR)
            dst = qT if m < 8 else kTl
            ve.tensor_scalar_add(dst[:, m % 8, :], ps[:],
                                 scalar1=bqkv_qk[:, m:m + 1])

        for m in range(8, 16):       # K first (per-t chases LN1 tiles)
            qk_tile(m, per_t=True)

        # V in natural token-major layout (shares the kTl/mT ring)
        v_sb = shp.tile([P, TT, D], F8, tag="sh", name="v_sb")
        for c in range(2):
            wvt = wv_p.tile([P, KD, 512], F8, tag="wv", name="wv")
            gp.dma_start(wvt[:], wqkv8[:, 2 * D + c * 512:2 * D + (c + 1) * 512]
                         .rearrange("(k p) m -> p k m", p=P))
            for t in range(TT):
                ps = mm_ps.tile([P, 512], F32, tag="mm", name="mm_v")
                for k2 in range(KD // 2):
                    te.matmul(ps[:], hT[:, 2 * k2:2 * k2 + 2,
                                        t * P:(t + 1) * P],
                              wvt[:, 2 * k2:2 * k2 + 2, :],
                              start=(k2 == 0), stop=(k2 == KD // 2 - 1),
                              perf_mode=DR)
                ve.tensor_add(v_sb[:, t, c * 512:(c + 1) * 512], ps[:],
                              bv_bc[:, c * 512:(c + 1) * 512])

        # single AllGather of K^T and V, launched as soon as both exist
        CCIN = 2 * KEL
        cc_in = dram.tile([CCIN], F8)
        sy.dma_start(cc_in[0:KEL].rearrange("(k p t) -> p k t", k=KD, p=P),
                     kTl[:])
        sy.dma_start(cc_in[KEL:CCIN].rearrange("(t p d) -> p t d", t=TT, p=P),
                     v_sb[:])
        cc_out = dram.tile([GS * CCIN], F8)
        gp.collective_compute(
            "AllGather", Alu.bypass, ins=[cc_in[:]], outs=[cc_out[:]],
            replica_groups=GROUPS)

        for m in range(0, 8):        # Q overlaps the ring
            qk_tile(m, per_t=False)

        # unpack K (two queues), then V
        kT_full = persist.tile([P, KD, GS, TOK], F8, tag="kT_full")
        for r in range(GS):
            eng = sy if r % 2 == 0 else gp
            eng.dma_start(
                kT_full[:, :, r, :],
                cc_out[r * CCIN:r * CCIN + KEL].rearrange(
                    "(k p t) -> p k t", k=KD, p=P))
        # v unpack has 64B-granular descriptors; spread over 4 engine queues
        v_aug = persist.tile([P, NKJ, H, 65], F8, tag="v_aug")
        ve.memset(v_aug[:, :, :, 64:65], 1.0)
        vengs = [sy, gp, sc]
        for r in range(GS):
            for vt in range(TT):
                eng = vengs[(r * TT + vt) % 3]
                base = r * CCIN + KEL + vt * P * D
                eng.dma_start(
                    v_aug[:, r * TT + vt, :, 0:64],
                    cc_out[base:base + P * D].rearrange(
                        "(p h f) -> p h f", p=P, h=H))

        # ------- phase 3: chunked attention + interleaved MLP ---------
        attnT = persist.tile([64, H, TOK], F8, tag="attnT")
        mT = [shp.tile([P, KD, CTOK], BF16, tag="sh", name=f"mT{c}")
              for c in range(CH)]
        g1T = persist.tile([P, FT, CTOK], BF16, tag="g1T")

        def attention_chunk(ch, units):
            q0 = ch * CTOK
            done = [0]
            step = [0]
            nsteps = 8 * (NKJ // 2)

            def pump():
                step[0] += 1
                want = (step[0] * len(units)) // nsteps if units else 0
                while done[0] < want:
                    units[done[0]]()
                    done[0] += 1

            pend = []   # (av, pb, j2) awaiting AV emission
            for pr in range(8):
                av = av_ps.tile([65, 2, CTOK], F32, tag="av", name="av")
                b32 = (pr % 2) * 64
                mt0 = 2 * (pr // 2)
                for j2 in range(NKJ // 2):
                    r, jj = divmod(2 * j2, TT)
                    sp = sp_ps.tile([P, 2, 2, CTOK], F32, tag="sp", name="sp")
                    for hpp in range(2):
                        bb = b32 + hpp * 32
                        for dj in range(2):
                            te.matmul(
                                sp[:, hpp, dj, :],
                                kT_full[bb:bb + 32, mt0:mt0 + 2, r,
                                        (jj + dj) * P:(jj + dj + 1) * P],
                                qT[bb:bb + 32, mt0:mt0 + 2, q0:q0 + CTOK],
                                start=True, stop=True, perf_mode=DR,
                                tile_position=(bb, 0))
                    pb = probs.tile([P, 2, 2, CTOK], F8, tag="probs",
                                    name="probs")
                    sc.activation(pb[:], sp[:], Act.Exp, scale=0.125)
                    pend.append((av, pb, j2))
                    if len(pend) > 1:
                        pav, ppb, pj2 = pend.pop(0)
                        for h2 in range(2):
                            te.matmul(pav[:, h2, :],
                                      v_aug[:, 2 * pj2:2 * pj2 + 2,
                                            2 * pr + h2, :],
                                      ppb[:, h2, :, :],
                                      start=(pj2 == 0), stop=False,
                                      perf_mode=DR)
                    pump()
                # last AV of the pair
                pav, ppb, pj2 = pend.pop(0)
                for h2 in range(2):
                    te.matmul(pav[:, h2, :],
                              v_aug[:, 2 * pj2:2 * pj2 + 2, 2 * pr + h2, :],
                              ppb[:, h2, :, :],
                              start=(pj2 == 0), stop=(pj2 == 7),
                              perf_mode=DR)
                # normalize: reciprocal of the ones-column accumulator
                dstg = norm.tile([65, 2, CTOK], F32, tag="dstg", name="dstg")
                ve.tensor_copy(dstg[64:65, :, :], av[64:65, :, :])
                den0 = norm.tile([1, 2, 2, CTOK], F32, tag="den0", name="den0")
                gp.dma_start(den0[:, 0, :, :], dstg[64:65, :, :])
                ve.reciprocal_approx_fast(den0[:, 1, :, :], den0[:, 0, :, :])
                rbc = norm.tile([64, 2, CTOK], F32, tag="rbc", name="rbc")
                gp.partition_broadcast(rbc[:], den0[:, 1, :, :])
                for h2 in range(2):
                    ve.tensor_mul(attnT[:, 2 * pr + h2, q0:q0 + CTOK],
                                  av[0:64, h2, :], rbc[:, h2, :])
            while done[0] < len(units):
                units[done[0]]()
                done[0] += 1

        def wo_chunk(ch):
            q0 = ch * CTOK
            for c2 in range(2):
                for qt in range(CQT):
                    ps = mm_ps.tile([P, 512], F32, tag="mm", name="mm_wo")
                    for pr in range(8):
                        te.matmul(ps[:],
                                  attnT[:, 2 * pr:2 * pr + 2,
                                        q0 + qt * P:q0 + (qt + 1) * P],
                                  wo8t[:, 2 * pr:2 * pr + 2,
                                       c2 * 512:(c2 + 1) * 512],
                                  start=(pr == 0), stop=(pr == 7),
                                  perf_mode=DR)
                    xsl = x1_sb[:, ch * CQT + qt, c2 * 512:(c2 + 1) * 512]
                    ve.tensor_add(xsl, xsl, ps[:])

        def ln2_chunk(ch):
            for qt in range(CQT):
                mt = act.tile([P, D], BF16, tag="hmt", name="mln")
                layer_norm(x1_sb[:, ch * CQT + qt, :], mt[:])
                for k in range(KD):
                    pe_t(mT[ch][:, k, qt * P:(qt + 1) * P],
                         mt[:, k * P:(k + 1) * P])

        def mlp_units(ch):
            units = []
            c = 0.7978845608028654

            def w1_unit(mp):
                def u():
                    ew = ve
                    w1c = wmlp.tile([P, KD, 2 * P], BF16, tag="w1c",
                                    name="w1c")
                    sy.dma_start(w1c[:], w1_ext[:, mp * 2 * P:(mp + 1) * 2 * P]
                                 .rearrange("(k p) m -> p k m", p=P))
                    ps = mm_ps.tile([P, 2, CTOK], F32, tag="mm", name="mm_w1")
                    for hf in range(2):
                        for k in range(KD):
                            te.matmul(ps[:, hf, :],
                                      w1c[:, k, hf * P:(hf + 1) * P],
                                      mT[ch][:, k, :],
                                      start=(k == 0), stop=(k == KD - 1))
                    xb = gelu.tile([P, 2, CTOK], BF16, tag="gxb", name="gxb")
                    for hf in range(2):
                        # PSUM read must stay on DVE (gpsimd can't touch PSUM)
                        ve.tensor_scalar_add(
                            xb[:, hf, :], ps[:, hf, :],
                            scalar1=b1col[:, 2 * mp + hf:2 * mp + hf + 1])
                    t1 = gelu.tile([P, 2, CTOK], BF16, tag="gt1", name="gt1")
                    ew.tensor_mul(t1[:], xb[:], xb[:])
                    ew.tensor_scalar(t1[:], t1[:], scalar1=0.044715,
                                     scalar2=1.0, op0=Alu.mult, op1=Alu.add)
                    ew.tensor_mul(t1[:], t1[:], xb[:])
                    th = gelu.tile([P, 2, CTOK], BF16, tag="gth", name="gth")
                    sc.activation(th[:], t1[:], Act.Tanh, scale=c)
                    ew.tensor_scalar(th[:], th[:], scalar1=0.5, scalar2=0.5,
                                     op0=Alu.mult, op1=Alu.add)
                    ew.tensor_mul(g1T[:, 2 * mp:2 * mp + 2, :], th[:], xb[:])
                return u

            for mp in range(FT // 2):
                units.append(w1_unit(mp))

            w2state = {}

            def w2_unit(c2, fg):
                def u():
                    if fg == 0:
                        w2state[c2] = [
                            mm_ps.tile([P, 512], F32, tag="mm",
                                       name="mm_w2")
                            for _ in range(CQT)]
                    pss = w2state[c2]
                    # one batched load covers 8 FF row-tiles
                    w2c = shw.tile([P, 8, 512], BF16, tag="shw", name="w2c")
                    f0 = fg * 8
                    sy.dma_start(w2c[:], w2_ext[f0 * P:(f0 + 8) * P,
                                                c2 * 512:(c2 + 1) * 512]
                                 .rearrange("(f p) c -> p f c", p=P))
                    for fi in range(8):
                        for qt in range(CQT):
                            te.matmul(pss[qt][:],
                                      g1T[:, f0 + fi, qt * P:(qt + 1) * P],
                                      w2c[:, fi, :],
                                      start=(f0 + fi == 0), stop=False)
                    if fg == 3:
                        for qt in range(CQT):
                            te.matmul(pss[qt][:], ones_row[:],
                                      b2row[:, c2 * 512:(c2 + 1) * 512],
                                      start=False, stop=True)
                            ot = act.tile([P, 512], F32, tag="oout",
                                          name="oout")
                            ve.tensor_add(
                                ot[:], pss[qt][:],
                                x1_sb[:, ch * CQT + qt,
                                      c2 * 512:(c2 + 1) * 512])
                            row = (ch * CQT + qt) * P
                            sy.dma_start(
                                out_ext[row:row + P,
                                        c2 * 512:(c2 + 1) * 512], ot[:])
                return u

            for c2 in range(2):
                for fg in range(4):
                    units.append(w2_unit(c2, fg))
            return units

        attention_chunk(0, [])
        wo_chunk(0)
        ln2_chunk(0)
        attention_chunk(1, mlp_units(0))
        wo_chunk(1)
        ln2_chunk(1)
        for u in mlp_units(1):
            u()

    nc.compile()
    return nc


def _get_nc():
    if "nc" not in _cache:
        _cache["nc"] = _build()
    return _cache["nc"]


def _prep_weights(inputs):
    import ml_dtypes
    F8 = ml_dtypes.float8_e4m3
    BF = ml_dtypes.bfloat16

    f = {k: np.asarray(inputs[k], dtype=np.float32) for k in (
        "ln1_g", "ln1_b", "Wqkv", "bqkv", "Wo", "bo",
        "ln2_g", "ln2_b", "W1", "b1", "W2", "b2")}

    Wqkv = f["ln1_g"][:, None] * f["Wqkv"]
    bqkv = f["bqkv"] + f["ln1_b"] @ f["Wqkv"]
    W1 = f["ln2_g"][:, None] * f["W1"]
    b1 = f["b1"] + f["ln2_b"] @ f["W1"]

    # Q/K column permutation for DoubleRow score layout
    d = np.arange(D)
    pr, rem = d // 128, d % 128
    hp, half, p32 = rem // 64, (rem % 64) // 32, d % 32
    newpos = (pr // 2 * 2 + half) * 128 + (pr % 2) * 64 + hp * 32 + p32
    wq_p = np.empty((D, 3 * D), np.float32)
    wq_p[:, newpos] = Wqkv[:, :D]
    wq_p[:, D + newpos] = Wqkv[:, D:2 * D]
    wq_p[:, 2 * D:] = Wqkv[:, 2 * D:]
    bq_p = np.empty(3 * D, np.float32)
    bq_p[newpos] = bqkv[:D]
    bq_p[D + newpos] = bqkv[D:2 * D]
    bq_p[2 * D:] = bqkv[2 * D:]

    return {
        "wqkv8": np.ascontiguousarray(wq_p.astype(F8)),
        "bqkvf": np.ascontiguousarray(bq_p),
        "wo8": np.ascontiguousarray(f["Wo"].astype(F8)),
        "bo": f["bo"],
        "w1f": np.ascontiguousarray(W1.astype(BF)),
        "b1f": np.ascontiguousarray(b1),
        "w2b": np.ascontiguousarray(f["W2"].astype(BF)),
        "b2": f["b2"],
    }


def make_in_maps(inputs):
    x = np.ascontiguousarray(np.asarray(inputs["x"], dtype=np.float32))
    flat = x.reshape(B * S, D)
    w = _prep_weights(inputs)
    in_maps = []
    for c in range(NCORES):
        m = {"x": np.ascontiguousarray(flat[c * TOK:(c + 1) * TOK])}
        m.update(w)
        in_maps.append(m)
    return in_maps


def kernel(**inputs):
    from concourse.bass_utils import run_bass_kernel_spmd

    nc = _get_nc()
    in_maps = make_in_maps(inputs)
    res = run_bass_kernel_spmd(nc, in_maps, core_ids=list(range(NCORES)))
    out = np.concatenate([res.results[c]["out"] for c in range(NCORES)],
                         axis=0)
    return out.reshape(B, S, D).astype(np.float32)


# revision 31
# speedup vs baseline: 1.0923x; 1.0383x over previous
"""Distributed Trainium2 kernel for a dense transformer block.

Sequence-parallel over 8 NeuronCores (512 tokens each; cores 0-3 hold
batch 0, 4-7 batch 1). Weights replicated. Two AllGathers (K then V)
within each 4-core group; everything else local.

Host-side preprocessing (in kernel()):
 - LayerNorm gains folded into the weights: Wqkv' = diag(ln1_g)@Wqkv,
   bqkv' = bqkv + ln1_b@Wqkv (same for W1/ln2), so the device only
   computes (x-mu)*rsqrt(var+eps).
 - Q/K columns permuted so the QKV matmul emits q/k directly in the
   [32-partition, dh-half-pair] layout DoubleRow score matmuls need.
 - Wqkv/Wo cast to fp8e4m3, W1/W2 to bf16 (attention path fp8 is
   ~1.9e-3 rel err; MLP must stay bf16).

Device-side structure:
 - All attention-path matmuls (QKV, scores, AV, Wo) run fp8 DoubleRow
   (two contraction tiles per instruction).
 - AV emits attn^T [dh, q] directly with a ones-column producing the
   softmax denominator; normalization happens before Wo via a
   partition-broadcast reciprocal. No DMA transposes.
 - exp and the hand-rolled tanh-form gelu share one ACT table
   (exp_and_others); LayerNorm rsqrt is Newton iteration on DVE, so
   the scalar engine never swaps tables after warmup.
 - Tokens are processed in two 256-token chunks: chunk1's softmax exp
   (scalar engine) overlaps chunk0's MLP (tensor engine) via
   interleaved emission.
"""

import sys

if "/opt/trn_rl_repo" not in sys.path:
    sys.path.insert(0, "/opt/trn_rl_repo")

import numpy as np

B, S, D = 2, 2048, 1024
H, DH, FF = 16, 64, 4096
NCORES = 8
TOK = (B * S) // NCORES      # 512 tokens per core
P = 128
TT = TOK // P                # 4 token tiles
KD = D // P                  # 8 contract tiles over D
FT = FF // P                 # 32 tiles over FF
GS = 4                       # group size (cores per batch)
NKJ = S // P                 # 16 key tiles per batch
CH = 2                       # token chunks for attn/MLP pipelining
CQT = TT // CH               # 2 token tiles per chunk
CTOK = TOK // CH             # 256 tokens per chunk
GROUPS = [[0, 1, 2, 3], [4, 5, 6, 7]]
KEL = D * TOK                # fp8 elements in K bounce buffer
VEL = D * TOK

_cache = {}


def _build():
    from contextlib import ExitStack
    from concourse import bacc, tile, mybir
    from concourse.masks import make_identity

    F32 = mybir.dt.float32
    BF16 = mybir.dt.bfloat16
    F8 = mybir.dt.float8e4
    Alu = mybir.AluOpType
    Act = mybir.ActivationFunctionType
    DR = mybir.MatmulPerfMode.DoubleRow

    nc = bacc.Bacc("TRN2", target_bir_lowering=False, debug=False,
                   num_devices=NCORES)

    x_ext = nc.dram_tensor("x", [TOK, D], F32, kind="ExternalInput")
    wqkv8 = nc.dram_tensor("wqkv8", [D, 3 * D], F8, kind="ExternalInput")
    bqkvf = nc.dram_tensor("bqkvf", [3 * D], F32, kind="ExternalInput")
    wo8_ext = nc.dram_tensor("wo8", [D, D], F8, kind="ExternalInput")
    bo_ext = nc.dram_tensor("bo", [D], F32, kind="ExternalInput")
    w1_ext = nc.dram_tensor("w1f", [D, FF], BF16, kind="ExternalInput")
    b1_ext = nc.dram_tensor("b1f", [FF], F32, kind="ExternalInput")
    w2_ext = nc.dram_tensor("w2b", [FF, D], BF16, kind="ExternalInput")
    b2_ext = nc.dram_tensor("b2", [D], F32, kind="ExternalInput")
    out_ext = nc.dram_tensor("out", [TOK, D], F32, kind="ExternalOutput")

    with tile.TileContext(nc) as tc, ExitStack() as ctx:
        const = ctx.enter_context(tc.tile_pool(name="const", bufs=1))
        persist = ctx.enter_context(tc.tile_pool(name="persist", bufs=1))
        wq = ctx.enter_context(tc.tile_pool(name="wq", bufs=6))
        wv_p = ctx.enter_context(tc.tile_pool(name="wv_p", bufs=2))
        shp = ctx.enter_context(tc.tile_pool(name="shp", bufs=2))
        shw = ctx.enter_context(tc.tile_pool(name="shw", bufs=2))
        wmlp = ctx.enter_context(tc.tile_pool(name="wmlp", bufs=3))
        act = ctx.enter_context(tc.tile_pool(name="act", bufs=3))
        probs = ctx.enter_context(tc.tile_pool(name="probs", bufs=3))
        gelu = ctx.enter_context(tc.tile_pool(name="gelu", bufs=2))
        norm = ctx.enter_context(tc.tile_pool(name="norm", bufs=1))
        mm_ps = ctx.enter_context(
            tc.tile_pool(name="mm_ps", bufs=3, space="PSUM"))
        sp_ps = ctx.enter_context(
            tc.tile_pool(name="sp_ps", bufs=2, space="PSUM"))
        av_ps = ctx.enter_context(
            tc.tile_pool(name="av_ps", bufs=1, space="PSUM"))
        dram = ctx.enter_context(tc.tile_pool(name="dram", bufs=1, space="DRAM"))

        sy, gp, ve, sc, te = nc.sync, nc.gpsimd, nc.vector, nc.scalar, nc.tensor

        # x lands first on the sync queue
        x1_sb = persist.tile([P, TT, D], F32, tag="x1")
        for t in range(TT):
            sy.dma_start(x1_sb[:, t, :], x_ext[t * P:(t + 1) * P, :])

        # ---------------- constants ----------------
        ones_row = const.tile([1, P], BF16)
        ve.memset(ones_row[:], 1.0)
        ident = const.tile([P, P], BF16)
        make_identity(nc, ident[:])

        # prewarm the exp/tanh ACT table off the critical path
        warm = const.tile([1, 2], F32)
        ve.memset(warm[:], 0.0)
        sc.activation(warm[:, 1:2], warm[:, 0:1], Act.Exp)

        bqkv_qk = const.tile([P, 16], F32)
        gp.dma_start(bqkv_qk[:], bqkvf[0:2 * D].rearrange("(m p) -> p m", p=P))
        b1col = const.tile([P, FT], F32)
        gp.dma_start(b1col[:], b1_ext[:].rearrange("(m p) -> p m", p=P))

        def brow(src, name, dt=F32):
            row = act.tile([1, D], F32, tag="crow", name=f"{name}_r")
            gp.dma_start(row[:], src.rearrange("(a d) -> a d", a=1))
            if dt == F32:
                return row
            rb = const.tile([1, D], dt, name=name)
            ve.tensor_copy(rb[:], row[:])
            return rb

        b2row = brow(b2_ext[:], "b2row", BF16)
        bo_row = brow(bo_ext[:], "bo_row")
        bv_row = brow(bqkvf[2 * D:3 * D], "bv_row")
        bo_bc = const.tile([P, D], F32, name="bo_bc")
        gp.partition_broadcast(bo_bc[:], bo_row[:])
        bv_bc = const.tile([P, D], F32, name="bv_bc")
        gp.partition_broadcast(bv_bc[:], bv_row[:])

        # Wo in [64-part, head, dout] layout for DoubleRow head pairs
        wo8t = persist.tile([64, H, D], F8, tag="wo8t")
        gp.dma_start(wo8t[:], wo8_ext[:, :].rearrange("(h p) c -> p h c", p=64))

        # ---------------- helpers ----------------
        def newton_rsqrt(y_ap, v_ap, tmp_ap):
            # y = rsqrt(v), v in ~[0.7, 1.4]: 3 Newton steps from y0=1
            ve.memset(y_ap, 1.0)
            for _ in range(3):
                ve.tensor_mul(tmp_ap, y_ap, y_ap)
                ve.tensor_mul(tmp_ap, tmp_ap, v_ap)
                ve.tensor_scalar(tmp_ap, tmp_ap, scalar1=-0.5, scalar2=1.5,
                                 op0=Alu.mult, op1=Alu.add)
                ve.tensor_mul(y_ap, y_ap, tmp_ap)

        def layer_norm(x_ap, out_ap):
            st = act.tile([P, 2, 6], F32, tag="ln_st", name="ln_st")
            ve.bn_stats(st[:, 0, :], x_ap[:, 0:512])
            ve.bn_stats(st[:, 1, :], x_ap[:, 512:1024])
            mv = act.tile([P, 2], F32, tag="ln_mv", name="ln_mv")
            ve.bn_aggr(mv[:], st[:])
            vv = act.tile([P, 3], F32, tag="ln_vy", name="ln_vy")
            ve.tensor_scalar_add(vv[:, 0:1], mv[:, 1:2], 1e-5)
            newton_rsqrt(vv[:, 1:2], vv[:, 0:1], vv[:, 2:3])
            ve.tensor_scalar(out_ap, x_ap, scalar1=mv[:, 0:1],
                             scalar2=vv[:, 1:2], op0=Alu.subtract,
                             op1=Alu.mult)

        def pe_t(dst_ap, src_ap):
            tp = mm_ps.tile([P, P], BF16, tag="mm", name="tp")
            te.transpose(tp[:], src_ap, ident[:])
            ve.tensor_copy(dst_ap, tp[:])

        # ---------------- phase 1: LN1 + transpose (fp8 hT) ----------
        hT = persist.tile([P, KD, TOK], F8, tag="hT")
        for t in range(TT):
            ht = act.tile([P, D], BF16, tag="hmt", name="hmt")
            layer_norm(x1_sb[:, t, :], ht[:])
            for k in range(KD):
                pe_t(hT[:, k, t * P:(t + 1) * P], ht[:, k * P:(k + 1) * P])
            # residual absorbs the Wo bias once, after LN1 consumed x
            ve.tensor_add(x1_sb[:, t, :], x1_sb[:, t, :], bo_bc[:])

        # ---------------- phase 2: K, AG(K), Q, V, AG(V) --------------
        qT = persist.tile([P, KD, TOK], F8, tag="qT")
        # kTl shares a 4KB/partition ring with the later mT chunk tiles
        kTl = shp.tile([P, KD, TOK], F8, tag="sh", name="kTl")

        def qk_tile(m, per_t):
            # one 128-column tile of the Q/K projection
            wqt = wq.tile([P, KD, P], F8, tag="wq", name="wq")
            gp.dma_start(wqt[:], wqkv8[:, m * P:(m + 1) * P].rearrange(
                "(k p) m -> p k m", p=P))
            ps = mm_ps.tile([P, TOK], F32, tag="mm", name="mm_qk")
            tranges = [(t * P, (t + 1) * P) for t in range(TT)] if per_t \
                else [(0, TOK)]
            for k in range(KD):
                for lo, hi in tranges:
                    te.matmul(ps[:, lo:hi], wqt[:, k, :],
                              hT[:, k, lo:hi],
                              start=(k == 0), stop=(k == KD - 1))
            dst = qT if m < 8 else kTl
            # PSUM->SBUF copy + bias on the (otherwise idle) scalar engine
            sc.activation(dst[:, m % 8, :], ps[:], Act.Identity,
                          bias=bqkv_qk[:, m:m + 1])

        for m in range(8, 16):       # K first (per-t chases LN1 tiles)
            qk_tile(m, per_t=True)

        # V in natural token-major layout (shares the kTl/mT ring)
        v_sb = shp.tile([P, TT, D], F8, tag="sh", name="v_sb")
        for c in range(2):
            wvt = wv_p.tile([P, KD, 512], F8, tag="wv", name="wv")
            gp.dma_start(wvt[:], wqkv8[:, 2 * D + c * 512:2 * D + (c + 1) * 512]
                         .rearrange("(k p) m -> p k m", p=P))
            for t in range(TT):
                ps = mm_ps.tile([P, 512], F32, tag="mm", name="mm_v")
                for k in range(KD):
                    te.matmul(ps[:], hT[:, k, t * P:(t + 1) * P],
                              wvt[:, k, :],
                              start=(k == 0), stop=(k == KD - 1))
                ve.tensor_add(v_sb[:, t, c * 512:(c + 1) * 512], ps[:],
                              bv_bc[:, c * 512:(c + 1) * 512])

        # single AllGather of K^T and V, launched as soon as both exist
        CCIN = 2 * KEL
        cc_in = dram.tile([CCIN], F8)
        sy.dma_start(cc_in[0:KEL].rearrange("(k p t) -> p k t", k=KD, p=P),
                     kTl[:])
        sy.dma_start(cc_in[KEL:CCIN].rearrange("(t p d) -> p t d", t=TT, p=P),
                     v_sb[:])
        cc_out = dram.tile([GS * CCIN], F8)
        gp.collective_compute(
            "AllGather", Alu.bypass, ins=[cc_in[:]], outs=[cc_out[:]],
            replica_groups=GROUPS)

        for m in range(0, 8):        # Q overlaps the ring
            qk_tile(m, per_t=False)

        # unpack K (two queues), then V
        kT_full = persist.tile([P, KD, GS, TOK], F8, tag="kT_full")
        for r in range(GS):
            eng = sy if r % 2 == 0 else gp
            eng.dma_start(
                kT_full[:, :, r, :],
                cc_out[r * CCIN:r * CCIN + KEL].rearrange(
                    "(k p t) -> p k t", k=KD, p=P))
        # v unpack has 64B-granular descriptors; spread over 4 engine queues
        v_aug = persist.tile([P, NKJ, H, 65], F8, tag="v_aug")
        ve.memset(v_aug[:, :, :, 64:65], 1.0)
        vengs = [sy, gp, sc]
        for r in range(GS):
            for vt in range(TT):
                eng = vengs[(r * TT + vt) % 3]
                base = r * CCIN + KEL + vt * P * D
                eng.dma_start(
                    v_aug[:, r * TT + vt, :, 0:64],
                    cc_out[base:base + P * D].rearrange(
                        "(p h f) -> p h f", p=P, h=H))

        # ------- phase 3: chunked attention + interleaved MLP ---------
        attnT = persist.tile([64, H, TOK], F8, tag="attnT")
        mT = [shp.tile([P, KD, CTOK], BF16, tag="sh", name=f"mT{c}")
              for c in range(CH)]
        g1T = persist.tile([P, FT, CTOK], BF16, tag="g1T")

        def attention_chunk(ch, units):
            q0 = ch * CTOK
            done = [0]
            step = [0]
            nsteps = 8 * (NKJ // 2)

            def pump():
                step[0] += 1
                want = (step[0] * len(units)) // nsteps if units else 0
                while done[0] < want:
                    units[done[0]]()
                    done[0] += 1

            def av_mms(av, pb, pr, j2, last):
                for h2 in range(2):
                    for dj in range(2):
                        te.matmul(av[:, h2, :],
                                  v_aug[:, 2 * j2 + dj, 2 * pr + h2, :],
                                  pb[:, h2, dj, :],
                                  start=(j2 == 0 and dj == 0),
                                  stop=(last and dj == 1))

            pend = []   # (av, pb, j2) awaiting AV emission
            for pr in range(8):
                av = av_ps.tile([65, 2, CTOK], F32, tag="av", name="av")
                for j2 in range(NKJ // 2):
                    r, jj = divmod(2 * j2, TT)
                    sp = sp_ps.tile([P, 2, 2, CTOK], F32, tag="sp", name="sp")
                    for hpp in range(2):
                        lo = hpp * 64
                        for dj in range(2):
                            te.matmul(
                                sp[:, hpp, dj, :],
                                kT_full[lo:lo + 64, pr, r,
                                        (jj + dj) * P:(jj + dj + 1) * P],
                                qT[lo:lo + 64, pr, q0:q0 + CTOK],
                                start=True, stop=True)
                    pb = probs.tile([P, 2, 2, CTOK], F8, tag="probs",
                                    name="probs")
                    sc.activation(pb[:], sp[:], Act.Exp, scale=0.125)
                    pend.append((av, pb, j2))
                    if len(pend) > 1:
                        pav, ppb, pj2 = pend.pop(0)
                        av_mms(pav, ppb, pr, pj2, last=False)
                    pump()
                # last AV of the pair
                pav, ppb, pj2 = pend.pop(0)
                av_mms(pav, ppb, pr, pj2, last=True)
                # normalize: reciprocal of the ones-column accumulator
                dstg = norm.tile([65, 2, CTOK], F32, tag="dstg", name="dstg")
                ve.tensor_copy(dstg[64:65, :, :], av[64:65, :, :])
                den0 = norm.tile([1, 2, 2, CTOK], F32, tag="den0", name="den0")
                gp.dma_start(den0[:, 0, :, :], dstg[64:65, :, :])
                ve.reciprocal_approx_fast(den0[:, 1, :, :], den0[:, 0, :, :])
                rbc = norm.tile([64, 2, CTOK], F32, tag="rbc", name="rbc")
                gp.partition_broadcast(rbc[:], den0[:, 1, :, :])
                for h2 in range(2):
                    ve.tensor_mul(attnT[:, 2 * pr + h2, q0:q0 + CTOK],
                                  av[0:64, h2, :], rbc[:, h2, :])
            while done[0] < len(units):
                units[done[0]]()
                done[0] += 1

        def wo_chunk(ch):
            q0 = ch * CTOK
            for c2 in range(2):
                for qt in range(CQT):
                    ps = mm_ps.tile([P, 512], F32, tag="mm", name="mm_wo")
                    for h in range(H):
                        te.matmul(ps[:],
                                  attnT[:, h,
                                        q0 + qt * P:q0 + (qt + 1) * P],
                                  wo8t[:, h, c2 * 512:(c2 + 1) * 512],
                                  start=(h == 0), stop=(h == H - 1))
                    xsl = x1_sb[:, ch * CQT + qt, c2 * 512:(c2 + 1) * 512]
                    ve.tensor_add(xsl, xsl, ps[:])

        def ln2_chunk(ch):
            for qt in range(CQT):
                mt = act.tile([P, D], BF16, tag="hmt", name="mln")
                layer_norm(x1_sb[:, ch * CQT + qt, :], mt[:])
                for k in range(KD):
                    pe_t(mT[ch][:, k, qt * P:(qt + 1) * P],
                         mt[:, k * P:(k + 1) * P])

        def mlp_units(ch):
            units = []
            c = 0.7978845608028654

            def w1_unit(mp):
                def u():
                    ew = ve
                    w1c = wmlp.tile([P, KD, 2 * P], BF16, tag="w1c",
                                    name="w1c")
                    sy.dma_start(w1c[:], w1_ext[:, mp * 2 * P:(mp + 1) * 2 * P]
                                 .rearrange("(k p) m -> p k m", p=P))
                    ps = mm_ps.tile([P, 2, CTOK], F32, tag="mm", name="mm_w1")
                    for hf in range(2):
                        for k in range(KD):
                            te.matmul(ps[:, hf, :],
                                      w1c[:, k, hf * P:(hf + 1) * P],
                                      mT[ch][:, k, :],
                                      start=(k == 0), stop=(k == KD - 1))
                    xb = gelu.tile([P, 2, CTOK], BF16, tag="gxb", name="gxb")
                    for hf in range(2):
                        # PSUM read must stay on DVE (gpsimd can't touch PSUM)
                        ve.tensor_scalar_add(
                            xb[:, hf, :], ps[:, hf, :],
                            scalar1=b1col[:, 2 * mp + hf:2 * mp + hf + 1])
                    t1 = gelu.tile([P, 2, CTOK], BF16, tag="gt1", name="gt1")
                    ew.tensor_mul(t1[:], xb[:], xb[:])
                    ew.tensor_scalar(t1[:], t1[:], scalar1=0.044715,
                                     scalar2=1.0, op0=Alu.mult, op1=Alu.add)
                    ew.tensor_mul(t1[:], t1[:], xb[:])
                    th = gelu.tile([P, 2, CTOK], BF16, tag="gth", name="gth")
                    sc.activation(th[:], t1[:], Act.Tanh, scale=c)
                    ew.tensor_scalar(th[:], th[:], scalar1=0.5, scalar2=0.5,
                                     op0=Alu.mult, op1=Alu.add)
                    ew.tensor_mul(g1T[:, 2 * mp:2 * mp + 2, :], th[:], xb[:])
                return u

            for mp in range(FT // 2):
                units.append(w1_unit(mp))

            w2state = {}

            def w2_unit(c2, fg):
                def u():
                    if fg == 0:
                        w2state[c2] = [
                            mm_ps.tile([P, 512], F32, tag="mm",
                                       name="mm_w2")
                            for _ in range(CQT)]
                    pss = w2state[c2]
                    # one batched load covers 8 FF row-tiles
                    w2c = shw.tile([P, 8, 512], BF16, tag="shw", name="w2c")
                    f0 = fg * 8
                    sy.dma_start(w2c[:], w2_ext[f0 * P:(f0 + 8) * P,
                                                c2 * 512:(c2 + 1) * 512]
                                 .rearrange("(f p) c -> p f c", p=P))
                    for fi in range(8):
                        for qt in range(CQT):
                            te.matmul(pss[qt][:],
                                      g1T[:, f0 + fi, qt * P:(qt + 1) * P],
                                      w2c[:, fi, :],
                                      start=(f0 + fi == 0), stop=False)
                    if fg == 3:
                        for qt in range(CQT):
                            te.matmul(pss[qt][:], ones_row[:],
                                      b2row[:, c2 * 512:(c2 + 1) * 512],
                                      start=False, stop=True)
                            ot = act.tile([P, 512], F32, tag="oout",
                                          name="oout")
                            ve.tensor_add(
                                ot[:], pss[qt][:],
                                x1_sb[:, ch * CQT + qt,
                                      c2 * 512:(c2 + 1) * 512])
                            row = (ch * CQT + qt) * P
                            sy.dma_start(
                                out_ext[row:row + P,
                                        c2 * 512:(c2 + 1) * 512], ot[:])
                return u

            for c2 in range(2):
                for fg in range(4):
                    units.append(w2_unit(c2, fg))
            return units

        attention_chunk(0, [])
        wo_chunk(0)
        ln2_chunk(0)
        attention_chunk(1, mlp_units(0))
        wo_chunk(1)
        ln2_chunk(1)
        for u in mlp_units(1):
            u()

    nc.compile()
    return nc


def _get_nc():
    if "nc" not in _cache:
        _cache["nc"] = _build()
    return _cache["nc"]


def _prep_weights(inputs):
    import ml_dtypes
    F8 = ml_dtypes.float8_e4m3
    BF = ml_dtypes.bfloat16

    f = {k: np.asarray(inputs[k], dtype=np.float32) for k in (
        "ln1_g", "ln1_b", "Wqkv", "bqkv", "Wo", "bo",
        "ln2_g", "ln2_b", "W1", "b1", "W2", "b2")}

    Wqkv = f["ln1_g"][:, None] * f["Wqkv"]
    bqkv = f["bqkv"] + f["ln1_b"] @ f["Wqkv"]
    W1 = f["ln2_g"][:, None] * f["W1"]
    b1 = f["b1"] + f["ln2_b"] @ f["W1"]

    return {
        "wqkv8": np.ascontiguousarray(Wqkv.astype(F8)),
        "bqkvf": np.ascontiguousarray(bqkv),
        "wo8": np.ascontiguousarray(f["Wo"].astype(F8)),
        "bo": f["bo"],
        "w1f": np.ascontiguousarray(W1.astype(BF)),
        "b1f": np.ascontiguousarray(b1),
        "w2b": np.ascontiguousarray(f["W2"].astype(BF)),
        "b2": f["b2"],
    }


def make_in_maps(inputs):
    x = np.ascontiguousarray(np.asarray(inputs["x"], dtype=np.float32))
    flat = x.reshape(B * S, D)
    w = _prep_weights(inputs)
    in_maps = []
    for c in range(NCORES):
        m = {"x": np.ascontiguousarray(flat[c * TOK:(c + 1) * TOK])}
        m.update(w)
        in_maps.append(m)
    return in_maps


def kernel(**inputs):
    from concourse.bass_utils import run_bass_kernel_spmd

    nc = _get_nc()
    in_maps = make_in_maps(inputs)
    res = run_bass_kernel_spmd(nc, in_maps, core_ids=list(range(NCORES)))
    out = np.concatenate([res.results[c]["out"] for c in range(NCORES)],
                         axis=0)
    return out.reshape(B, S, D).astype(np.float32)


# revision 40
# speedup vs baseline: 1.2266x; 1.1230x over previous
"""Distributed Trainium2 kernel for a dense transformer block.

Sequence-parallel over 8 NeuronCores (512 tokens each; cores 0-3 hold
batch 0, 4-7 batch 1). Weights replicated. Two AllGathers (K then V)
within each 4-core group; everything else local.

Host-side preprocessing (in kernel()):
 - LayerNorm gains folded into the weights: Wqkv' = diag(ln1_g)@Wqkv,
   bqkv' = bqkv + ln1_b@Wqkv (same for W1/ln2), so the device only
   computes (x-mu)*rsqrt(var+eps).
 - Q/K columns permuted so the QKV matmul emits q/k directly in the
   [32-partition, dh-half-pair] layout DoubleRow score matmuls need.
 - Wqkv/Wo cast to fp8e4m3, W1/W2 to bf16 (attention path fp8 is
   ~1.9e-3 rel err; MLP must stay bf16).

Device-side structure:
 - All attention-path matmuls (QKV, scores, AV, Wo) run fp8 DoubleRow
   (two contraction tiles per instruction).
 - AV emits attn^T [dh, q] directly with a ones-column producing the
   softmax denominator; normalization happens before Wo via a
   partition-broadcast reciprocal. No DMA transposes.
 - exp and the hand-rolled tanh-form gelu share one ACT table
   (exp_and_others); LayerNorm rsqrt is Newton iteration on DVE, so
   the scalar engine never swaps tables after warmup.
 - Tokens are processed in two 256-token chunks: chunk1's softmax exp
   (scalar engine) overlaps chunk0's MLP (tensor engine) via
   interleaved emission.
"""

import sys

if "/opt/trn_rl_repo" not in sys.path:
    sys.path.insert(0, "/opt/trn_rl_repo")

import numpy as np

B, S, D = 2, 2048, 1024
H, DH, FF = 16, 64, 4096
NCORES = 8
TOK = (B * S) // NCORES      # 512 tokens per core
P = 128
TT = TOK // P                # 4 token tiles
KD = D // P                  # 8 contract tiles over D
FT = FF // P                 # 32 tiles over FF
GS = 4                       # group size (cores per batch)
NKJ = S // P                 # 16 key tiles per batch
CH = 2                       # token chunks for attn/MLP pipelining
CQT = TT // CH               # 2 token tiles per chunk
CTOK = TOK // CH             # 256 tokens per chunk
GROUPS = [[0, 1, 2, 3], [4, 5, 6, 7]]
KEL = D * TOK                # fp8 elements in K bounce buffer
VEL = D * TOK

_cache = {}


def _build():
    from contextlib import ExitStack
    from concourse import bacc, tile, mybir
    from concourse.masks import make_identity

    F32 = mybir.dt.float32
    BF16 = mybir.dt.bfloat16
    F8 = mybir.dt.float8e4
    Alu = mybir.AluOpType
    Act = mybir.ActivationFunctionType
    DR = mybir.MatmulPerfMode.DoubleRow

    nc = bacc.Bacc("TRN2", target_bir_lowering=False, debug=False,
                   num_devices=NCORES)

    x_ext = nc.dram_tensor("x", [TOK, D], F32, kind="ExternalInput")
    wqkv8 = nc.dram_tensor("wqkv8", [D, 3 * D], F8, kind="ExternalInput")
    bqkvf = nc.dram_tensor("bqkvf", [3 * D], F32, kind="ExternalInput")
    wo8_ext = nc.dram_tensor("wo8", [D, D], F8, kind="ExternalInput")
    bo_ext = nc.dram_tensor("bo", [D], F32, kind="ExternalInput")
    w1_ext = nc.dram_tensor("w1f", [D, FF], BF16, kind="ExternalInput")
    b1_ext = nc.dram_tensor("b1f", [FF], F32, kind="ExternalInput")
    w2_ext = nc.dram_tensor("w2b", [FF, D], BF16, kind="ExternalInput")
    b2_ext = nc.dram_tensor("b2", [D], F32, kind="ExternalInput")
    out_ext = nc.dram_tensor("out", [TOK, D], F32, kind="ExternalOutput")

    with tile.TileContext(nc) as tc, ExitStack() as ctx:
        const = ctx.enter_context(tc.tile_pool(name="const", bufs=1))
        persist = ctx.enter_context(tc.tile_pool(name="persist", bufs=1))
        wq = ctx.enter_context(tc.tile_pool(name="wq", bufs=6))
        wv_p = ctx.enter_context(tc.tile_pool(name="wv_p", bufs=2))
        shp = ctx.enter_context(tc.tile_pool(name="shp", bufs=2))
        shw = ctx.enter_context(tc.tile_pool(name="shw", bufs=2))
        wmlp = ctx.enter_context(tc.tile_pool(name="wmlp", bufs=3))
        act = ctx.enter_context(tc.tile_pool(name="act", bufs=3))
        probs = ctx.enter_context(tc.tile_pool(name="probs", bufs=3))
        gelu = ctx.enter_context(tc.tile_pool(name="gelu", bufs=2))
        norm = ctx.enter_context(tc.tile_pool(name="norm", bufs=1))
        mm_ps = ctx.enter_context(
            tc.tile_pool(name="mm_ps", bufs=3, space="PSUM"))
        sp_ps = ctx.enter_context(
            tc.tile_pool(name="sp_ps", bufs=2, space="PSUM"))
        av_ps = ctx.enter_context(
            tc.tile_pool(name="av_ps", bufs=1, space="PSUM"))
        dram = ctx.enter_context(tc.tile_pool(name="dram", bufs=1, space="DRAM"))

        sy, gp, ve, sc, te = nc.sync, nc.gpsimd, nc.vector, nc.scalar, nc.tensor

        # x lands first on the sync queue
        x1_sb = persist.tile([P, TT, D], F32, tag="x1")
        for t in range(TT):
            sy.dma_start(x1_sb[:, t, :], x_ext[t * P:(t + 1) * P, :])

        # ---------------- constants ----------------
        ones_row = const.tile([1, P], BF16)
        ve.memset(ones_row[:], 1.0)
        ident = const.tile([P, P], BF16)
        make_identity(nc, ident[:])

        # prewarm the exp/tanh ACT table off the critical path
        warm = const.tile([1, 2], F32)
        ve.memset(warm[:], 0.0)
        sc.activation(warm[:, 1:2], warm[:, 0:1], Act.Exp)

        bqkv_qk = const.tile([P, 16], F32)
        gp.dma_start(bqkv_qk[:], bqkvf[0:2 * D].rearrange("(m p) -> p m", p=P))
        b1col = const.tile([P, FT], F32)
        gp.dma_start(b1col[:], b1_ext[:].rearrange("(m p) -> p m", p=P))

        def brow(src, name, dt=F32):
            row = act.tile([1, D], F32, tag="crow", name=f"{name}_r")
            gp.dma_start(row[:], src.rearrange("(a d) -> a d", a=1))
            if dt == F32:
                return row
            rb = const.tile([1, D], dt, name=name)
            ve.tensor_copy(rb[:], row[:])
            return rb

        b2row = brow(b2_ext[:], "b2row", BF16)
        bo_row = brow(bo_ext[:], "bo_row")
        bv_row = brow(bqkvf[2 * D:3 * D], "bv_row")
        bo_bc = const.tile([P, D], F32, name="bo_bc")
        gp.partition_broadcast(bo_bc[:], bo_row[:])
        bv_bc = const.tile([P, D], F32, name="bv_bc")
        gp.partition_broadcast(bv_bc[:], bv_row[:])

        # Wo in [64-part, head, dout] layout; loaded via the scalar queue so
        # it never delays the Q/K/V weight stream on gpsimd
        wo8t = persist.tile([64, H, D], F8, tag="wo8t")
        sc.dma_start(wo8t[:], wo8_ext[:, :].rearrange("(h p) c -> p h c", p=64))

        # ---------------- helpers ----------------
        def newton_rsqrt(y_ap, v_ap, tmp_ap):
            # y = rsqrt(v), v in ~[0.7, 1.4]: 3 Newton steps from y0=1
            ve.memset(y_ap, 1.0)
            for _ in range(3):
                ve.tensor_mul(tmp_ap, y_ap, y_ap)
                ve.tensor_mul(tmp_ap, tmp_ap, v_ap)
                ve.tensor_scalar(tmp_ap, tmp_ap, scalar1=-0.5, scalar2=1.5,
                                 op0=Alu.mult, op1=Alu.add)
                ve.tensor_mul(y_ap, y_ap, tmp_ap)

        def layer_norm(x_ap, out_ap):
            st = act.tile([P, 2, 6], F32, tag="ln_st", name="ln_st")
            ve.bn_stats(st[:, 0, :], x_ap[:, 0:512])
            ve.bn_stats(st[:, 1, :], x_ap[:, 512:1024])
            mv = act.tile([P, 2], F32, tag="ln_mv", name="ln_mv")
            ve.bn_aggr(mv[:], st[:])
            vv = act.tile([P, 3], F32, tag="ln_vy", name="ln_vy")
            ve.tensor_scalar_add(vv[:, 0:1], mv[:, 1:2], 1e-5)
            newton_rsqrt(vv[:, 1:2], vv[:, 0:1], vv[:, 2:3])
            ve.tensor_scalar(out_ap, x_ap, scalar1=mv[:, 0:1],
                             scalar2=vv[:, 1:2], op0=Alu.subtract,
                             op1=Alu.mult)

        def pe_t(dst_ap, src_ap, eng=None):
            tp = mm_ps.tile([P, P], BF16, tag="mm", name="tp")
            te.transpose(tp[:], src_ap, ident[:])
            if eng is sc:
                sc.activation(dst_ap, tp[:], Act.Copy)
            else:
                ve.tensor_copy(dst_ap, tp[:])

        # ---------------- phase 1: LN1 + transpose (fp8 hT) ----------
        hT = persist.tile([P, KD, TOK], F8, tag="hT")
        for t in range(TT):
            ht = act.tile([P, D], BF16, tag="hmt", name="hmt")
            layer_norm(x1_sb[:, t, :], ht[:])
            for k in range(KD):
                # PSUM->SBUF copies on the idle scalar engine, freeing DVE
                pe_t(hT[:, k, t * P:(t + 1) * P], ht[:, k * P:(k + 1) * P],
                     eng=sc)
            # residual absorbs the Wo bias once, after LN1 consumed x
            ve.tensor_add(x1_sb[:, t, :], x1_sb[:, t, :], bo_bc[:])

        # ---------------- phase 2: K, AG(K), Q, V, AG(V) --------------
        qT = persist.tile([P, KD, TOK], F8, tag="qT")
        # kTl shares a 4KB/partition ring with the later mT chunk tiles
        kTl = shp.tile([P, KD, TOK], F8, tag="sh", name="kTl")

        def qk_block(mp, per_t):
            # two 128-column tiles of the Q/K projection per weight load
            wqt = wq.tile([P, KD, 2 * P], F8, tag="wq", name="wq")
            gp.dma_start(wqt[:], wqkv8[:, mp * 2 * P:(mp + 1) * 2 * P]
                         .rearrange("(k p) m -> p k m", p=P))
            for hf in range(2):
                m = 2 * mp + hf
                ps = mm_ps.tile([P, TOK], F32, tag="mm", name="mm_qk")
                tranges = [(t * P, (t + 1) * P) for t in range(TT)] if per_t \
                    else [(0, TOK)]
                for k in range(KD):
                    for lo, hi in tranges:
                        te.matmul(ps[:, lo:hi], wqt[:, k, hf * P:(hf + 1) * P],
                                  hT[:, k, lo:hi],
                                  start=(k == 0), stop=(k == KD - 1))
                dst = qT if m < 8 else kTl
                # PSUM->SBUF copy + bias on the idle scalar engine
                sc.activation(dst[:, m % 8, :], ps[:], Act.Identity,
                              bias=bqkv_qk[:, m:m + 1])

        for mp in range(4, 8):       # K first (per-t chases LN1 tiles)
            qk_block(mp, per_t=True)

        # V with the softmax-denominator ones column baked in BEFORE the
        # collective, so the post-gather unpack is a contiguous copy
        VROW = H * 65
        v_sb = persist.tile([P, TT, H, 65], F8, tag="v_sb")
        ve.memset(v_sb[:, :, :, 64:65], 1.0)
        for c in range(2):
            wvt = wv_p.tile([P, KD, 512], F8, tag="wv", name="wv")
            gp.dma_start(wvt[:], wqkv8[:, 2 * D + c * 512:2 * D + (c + 1) * 512]
                         .rearrange("(k p) m -> p k m", p=P))
            for t in range(TT):
                ps = mm_ps.tile([P, 512], F32, tag="mm", name="mm_v")
                for k in range(KD):
                    te.matmul(ps[:], hT[:, k, t * P:(t + 1) * P],
                              wvt[:, k, :],
                              start=(k == 0), stop=(k == KD - 1))
                ve.tensor_add(
                    v_sb[:, t, 8 * c:8 * (c + 1), 0:64],
                    ps[:].rearrange("p (h f) -> p h f", h=8),
                    bv_bc[:, c * 512:(c + 1) * 512]
                    .rearrange("p (h f) -> p h f", h=8))

        # single AllGather of K^T and V(+ones)
        VEL2 = TT * P * VROW
        CCIN = KEL + VEL2
        cc_in = dram.tile([CCIN], F8)
        sy.dma_start(cc_in[0:KEL].rearrange("(k p t) -> p k t", k=KD, p=P),
                     kTl[:])
        sy.dma_start(cc_in[KEL:CCIN].rearrange("(t p x) -> p t x", t=TT, p=P),
                     v_sb[:].rearrange("p t h f -> p t (h f)"))
        cc_out = dram.tile([GS * CCIN], F8)
        gp.collective_compute(
            "AllGather", Alu.bypass, ins=[cc_in[:]], outs=[cc_out[:]],
            replica_groups=GROUPS)

        for mp in range(0, 4):       # Q overlaps the ring
            qk_block(mp, per_t=False)

        # unpack: all transfers are contiguous-row copies now
        kT_full = persist.tile([P, KD, GS, TOK], F8, tag="kT_full")
        v_aug = persist.tile([P, NKJ, H, 65], F8, tag="v_aug")
        for r in range(GS):
            eng = sy if r % 2 == 0 else gp
            eng.dma_start(
                kT_full[:, :, r, :],
                cc_out[r * CCIN:r * CCIN + KEL].rearrange(
                    "(k p t) -> p k t", k=KD, p=P))
            base = r * CCIN + KEL
            eng.dma_start(
                v_aug[:, r * TT:(r + 1) * TT, :, :]
                .rearrange("p t h f -> p t (h f)"),
                cc_out[base:base + VEL2].rearrange(
                    "(t p x) -> p t x", t=TT, p=P))

        # ------- phase 3: chunked attention + interleaved MLP ---------
        attnT = persist.tile([64, H, TOK], F8, tag="attnT")
        mT = [shp.tile([P, KD, CTOK], BF16, tag="sh", name=f"mT{c}")
              for c in range(CH)]
        g1T = persist.tile([P, FT, CTOK], BF16, tag="g1T")

        def attention_chunk(ch, units):
            q0 = ch * CTOK
            done = [0]
            step = [0]
            nsteps = 8 * (NKJ // 2)

            def pump():
                step[0] += 1
                want = (step[0] * len(units)) // nsteps if units else 0
                while done[0] < want:
                    units[done[0]]()
                    done[0] += 1

            def av_mms(av, pb, pr, j2, last):
                for h2 in range(2):
                    for dj in range(2):
                        te.matmul(av[:, h2, :],
                                  v_aug[:, 2 * j2 + dj, 2 * pr + h2, :],
                                  pb[:, h2, dj, :],
                                  start=(j2 == 0 and dj == 0),
                                  stop=(last and dj == 1))

            pend = []   # (av, pb, j2) awaiting AV emission
            for pr in range(8):
                av = av_ps.tile([65, 2, CTOK], F32, tag="av", name="av")
                for j2 in range(NKJ // 2):
                    r, jj = divmod(2 * j2, TT)
                    sp = sp_ps.tile([P, 2, 2, CTOK], F32, tag="sp", name="sp")
                    for hpp in range(2):
                        lo = hpp * 64
                        for dj in range(2):
                            te.matmul(
                                sp[:, hpp, dj, :],
                                kT_full[lo:lo + 64, pr, r,
                                        (jj + dj) * P:(jj + dj + 1) * P],
                                qT[lo:lo + 64, pr, q0:q0 + CTOK],
                                start=True, stop=True)
                    pb = probs.tile([P, 2, 2, CTOK], F8, tag="probs",
                                    name="probs")
                    sc.activation(pb[:], sp[:], Act.Exp, scale=0.125)
                    pend.append((av, pb, j2))
                    if len(pend) > 1:
                        pav, ppb, pj2 = pend.pop(0)
                        av_mms(pav, ppb, pr, pj2, last=False)
                    pump()
                # last AV of the pair
                pav, ppb, pj2 = pend.pop(0)
                av_mms(pav, ppb, pr, pj2, last=True)
                # normalize: one quick full copy releases the PSUM bank,
                # then the reciprocal/broadcast chain works from SBUF
                dstg = norm.tile([65, 2, CTOK], F32, tag="dstg", name="dstg")
                ve.tensor_copy(dstg[:], av[:])
                den0 = norm.tile([1, 2, 2, CTOK], F32, tag="den0", name="den0")
                gp.dma_start(den0[:, 0, :, :], dstg[64:65, :, :])
                ve.reciprocal_approx_fast(den0[:, 1, :, :], den0[:, 0, :, :])
                rbc = norm.tile([64, 2, CTOK], F32, tag="rbc", name="rbc")
                gp.partition_broadcast(rbc[:], den0[:, 1, :, :])
                for h2 in range(2):
                    ve.tensor_mul(attnT[:, 2 * pr + h2, q0:q0 + CTOK],
                                  dstg[0:64, h2, :], rbc[:, h2, :])
            while done[0] < len(units):
                units[done[0]]()
                done[0] += 1

        def wo_chunk(ch):
            q0 = ch * CTOK
            for c2 in range(2):
                for qt in range(CQT):
                    ps = mm_ps.tile([P, 512], F32, tag="mm", name="mm_wo")
                    for h in range(H):
                        te.matmul(ps[:],
                                  attnT[:, h,
                                        q0 + qt * P:q0 + (qt + 1) * P],
                                  wo8t[:, h, c2 * 512:(c2 + 1) * 512],
                                  start=(h == 0), stop=(h == H - 1))
                    xsl = x1_sb[:, ch * CQT + qt, c2 * 512:(c2 + 1) * 512]
                    ve.tensor_add(xsl, xsl, ps[:])

        def ln2_chunk(ch):
            for qt in range(CQT):
                mt = act.tile([P, D], BF16, tag="hmt", name="mln")
                layer_norm(x1_sb[:, ch * CQT + qt, :], mt[:])
                for k in range(KD):
                    pe_t(mT[ch][:, k, qt * P:(qt + 1) * P],
                         mt[:, k * P:(k + 1) * P])

        def mlp_units(ch):
            units = []
            c = 0.7978845608028654

            weng = sy if ch == 0 else gp

            def w1_unit(mp):
                def u():
                    ew = ve
                    w1c = wmlp.tile([P, KD, 2 * P], BF16, tag="w1c",
                                    name="w1c")
                    weng.dma_start(w1c[:],
                                   w1_ext[:, mp * 2 * P:(mp + 1) * 2 * P]
                                   .rearrange("(k p) m -> p k m", p=P))
                    ps = mm_ps.tile([P, 2, CTOK], F32, tag="mm", name="mm_w1")
                    for hf in range(2):
                        for k in range(KD):
                            te.matmul(ps[:, hf, :],
                                      w1c[:, k, hf * P:(hf + 1) * P],
                                      mT[ch][:, k, :],
                                      start=(k == 0), stop=(k == KD - 1))
                    xb = gelu.tile([P, 2, CTOK], BF16, tag="gxb", name="gxb")
                    for hf in range(2):
                        # PSUM read must stay on DVE (gpsimd can't touch PSUM)
                        ve.tensor_scalar_add(
                            xb[:, hf, :], ps[:, hf, :],
                            scalar1=b1col[:, 2 * mp + hf:2 * mp + hf + 1])
                    t1 = gelu.tile([P, 2, CTOK], BF16, tag="gt1", name="gt1")
                    ew.tensor_mul(t1[:], xb[:], xb[:])
                    ew.tensor_scalar(t1[:], t1[:], scalar1=0.044715,
                                     scalar2=1.0, op0=Alu.mult, op1=Alu.add)
                    ew.tensor_mul(t1[:], t1[:], xb[:])
                    th = gelu.tile([P, 2, CTOK], BF16, tag="gt1", name="gth")
                    sc.activation(th[:], t1[:], Act.Tanh, scale=c)
                    ew.tensor_scalar(th[:], th[:], scalar1=0.5, scalar2=0.5,
                                     op0=Alu.mult, op1=Alu.add)
                    ew.tensor_mul(g1T[:, 2 * mp:2 * mp + 2, :], th[:], xb[:])
                return u

            for mp in range(FT // 2):
                units.append(w1_unit(mp))

            w2state = {}

            def w2_unit(c2, fg):
                def u():
                    if fg == 0:
                        w2state[c2] = [
                            mm_ps.tile([P, 512], F32, tag="mm",
                                       name="mm_w2")
                            for _ in range(CQT)]
                    pss = w2state[c2]
                    # one batched load covers 8 FF row-tiles
                    w2c = shw.tile([P, 8, 512], BF16, tag="shw", name="w2c")
                    f0 = fg * 8
                    sy.dma_start(w2c[:], w2_ext[f0 * P:(f0 + 8) * P,
                                                c2 * 512:(c2 + 1) * 512]
                                 .rearrange("(f p) c -> p f c", p=P))
                    _ = weng  # W2 stays on sync; W1/stores move in the tail
                    for fi in range(8):
                        for qt in range(CQT):
                            te.matmul(pss[qt][:],
                                      g1T[:, f0 + fi, qt * P:(qt + 1) * P],
                                      w2c[:, fi, :],
                                      start=(f0 + fi == 0), stop=False)
                    if fg == 3:
                        for qt in range(CQT):
                            te.matmul(pss[qt][:], ones_row[:],
                                      b2row[:, c2 * 512:(c2 + 1) * 512],
                                      start=False, stop=True)
                            ot = act.tile([P, 512], F32, tag="oout",
                                          name="oout")
                            ve.tensor_add(
                                ot[:], pss[qt][:],
                                x1_sb[:, ch * CQT + qt,
                                      c2 * 512:(c2 + 1) * 512])
                            row = (ch * CQT + qt) * P
                            weng.dma_start(
                                out_ext[row:row + P,
                                        c2 * 512:(c2 + 1) * 512], ot[:])
                return u

            for c2 in range(2):
                for fg in range(4):
                    units.append(w2_unit(c2, fg))
            return units

        attention_chunk(0, [])
        wo_chunk(0)
        ln2_chunk(0)
        attention_chunk(1, mlp_units(0))
        wo_chunk(1)
        ln2_chunk(1)
        for u in mlp_units(1):
            u()

    nc.compile()
    return nc


def _get_nc():
    if "nc" not in _cache:
        _cache["nc"] = _build()
    return _cache["nc"]


def _prep_weights(inputs):
    import ml_dtypes
    F8 = ml_dtypes.float8_e4m3
    BF = ml_dtypes.bfloat16

    f = {k: np.asarray(inputs[k], dtype=np.float32) for k in (
        "ln1_g", "ln1_b", "Wqkv", "bqkv", "Wo", "bo",
        "ln2_g", "ln2_b", "W1", "b1", "W2", "b2")}

    Wqkv = f["ln1_g"][:, None] * f["Wqkv"]
    bqkv = f["bqkv"] + f["ln1_b"] @ f["Wqkv"]
    W1 = f["ln2_g"][:, None] * f["W1"]
    b1 = f["b1"] + f["ln2_b"] @ f["W1"]

    return {
        "wqkv8": np.ascontiguousarray(Wqkv.astype(F8)),
        "bqkvf": np.ascontiguousarray(bqkv),
        "wo8": np.ascontiguousarray(f["Wo"].astype(F8)),
        "bo": f["bo"],
        "w1f": np.ascontiguousarray(W1.astype(BF)),
        "b1f": np.ascontiguousarray(b1),
        "w2b": np.ascontiguousarray(f["W2"].astype(BF)),
        "b2": f["b2"],
    }


def make_in_maps(inputs):
    x = np.ascontiguousarray(np.asarray(inputs["x"], dtype=np.float32))
    flat = x.reshape(B * S, D)
    w = _prep_weights(inputs)
    in_maps = []
    for c in range(NCORES):
        m = {"x": np.ascontiguousarray(flat[c * TOK:(c + 1) * TOK])}
        m.update(w)
        in_maps.append(m)
    return in_maps


def kernel(**inputs):
    from concourse.bass_utils import run_bass_kernel_spmd

    nc = _get_nc()
    in_maps = make_in_maps(inputs)
    res = run_bass_kernel_spmd(nc, in_maps, core_ids=list(range(NCORES)))
    out = np.concatenate([res.results[c]["out"] for c in range(NCORES)],
                         axis=0)
    return out.reshape(B, S, D).astype(np.float32)
